# revision 8
# baseline (speedup 1.0000x reference)
"""2-layer GCN on 8 trn2 cores — src-partitioned with ReduceScatter.

Per layer (h' = (x@W)*dis, dis = (deg+1)^-1/2):
  1. Each core computes h' for its OWN 12500 nodes (x@W on PE), writes the
     local table (fp8-e3m4, scaled by a power of 2 to sit in e3m4's normal
     range; rows padded to 256B for the gather stride) to DRAM.
  2. Each core gathers h'[src] rows for its own ~200k out-edges (sorted by
     global dst block) with big merged dma_gather ops, and reduces them into
     per-dst-block partial sums via one-hot matmuls accumulated in PSUM.
     One-hot masks are built per chunk with tensor_scalar is_equal (runs in
     the DVE 4x_2p fast mode).
  3. Partial tables (bf16) are summed across cores with piecewise
     ReduceScatters that overlap the sweep; per-piece epilogues are issued
     late (after the RS landed) so they interleave with the sweep without
     stalling engine queues.
  4. Epilogue: out = relu(dis*(agg + h'_self)); fp8 scale factors are folded
     into host-provided dis vectors and W2, so no extra unscale ops exist.

Edges live on the core that owns their SOURCE node, so the gather reads are
all core-local (int16 row ids fit) and the only collectives are the
(cheap, output-sized) ReduceScatters.  Host-side, nodes are packed into
blocks with an 8-dim greedy balancer so each (core, dst-block) cell fits in
2 chunks of 128 edge slots.
"""

import math

import ml_dtypes
import numpy as np

import concourse.bass as bass
import concourse.mybir as mybir
import concourse.tile as tile
from concourse.bass_utils import run_bass_kernel_spmd
from concourse.library_config import mlp
from concourse.masks import make_identity
from concourse.vector_clock import ScopedClock

P = 128
NCORES = 8
F32 = mybir.dt.float32
BF16 = mybir.dt.bfloat16
FP8 = mybir.dt.float8e3  # e3m4
I32 = mybir.dt.int32
I16 = mybir.dt.int16
PAD_LANE = 1000.0
GC = 8  # chunks per dma_gather instruction (1024 idx = SWDGE ring cap)
S1 = 4.0  # layer-1 fp8 table scale (folded into disT and W2)
S2 = 16.0  # layer-2 fp8 table scale (folded into dissqT and disO)


def _patched_drain_and_barrier(self, tick_clock, wait_clock):
    # This walrus build rejects >1 sem wait on TPB_CTRL (Drain) instructions.
    drain_inst = self.nc.sync.drain()
    wait_clock.add_sem_waits(
        drain_inst.ins, ScopedClock({None: tick_clock.global_clock})
    )
    si = drain_inst.ins.sync_info
    waits = list(si.on_wait)
    if len(waits) > 1:
        while len(si.on_wait):
            si.on_wait.pop()
        si.on_wait.append(waits[0])
        for w in waits[1:]:
            d2 = self.nc.sync.drain(fusable=False)
            si2 = d2.ins.sync_info
            if si2 is None:
                d2.ins.sync_info = mybir.SyncInfo(on_wait=[w], on_update=[])
            else:
                si2.on_wait.append(w)
    self.nc.all_engine_barrier()
    popped = self.nc._tile_sem_poison_stack.pop()
    assert popped is self._sem_poison
    self.nc.clear_and_free_semaphores(list(self.sems.allocated().values()))
    self.nc.all_engine_barrier()


tile.TileContext._drain_and_barrier = _patched_drain_and_barrier


def _spill_waits(nc, max_waits=1):
    """Move extra sync waits onto dedicated single-wait NoOps (walrus limit)."""
    n = 0
    for f in nc.m.functions:
        for blk in f.blocks:
            il = blk.instructions
            out = []
            for inst in il:
                si = inst.sync_info
                if si is not None and len(si.on_wait) > max_waits:
                    waits = list(si.on_wait)
                    while len(si.on_wait):
                        si.on_wait.pop()
                    for w in waits[:max_waits]:
                        si.on_wait.append(w)
                    for w in waits[max_waits:]:
                        nop = mybir.InstNoOp(
                            name=f"waitspill-{n}",
                            sync_info=mybir.SyncInfo(on_wait=[w], on_update=[]),
                            bass_nofuse=True,
                            engine=inst.engine,
                        )
                        n += 1
                        out.append(nop)
                out.append(inst)
            blk.instructions = out
    return n


def _dma_gather_raw(
    gp, out_ap, in_ap, idxs_ap, num_idxs, num_idxs_reg, elem_size, elem_step,
    queue_num=0,
):
    """dma_gather (non-transpose, DRAM source) without the elem%256B assert:
    elem_step (row stride) must be a 256B multiple, elem_size may be a
    fraction of a row.  Descriptors move elem_size*dtype bytes each."""
    from concourse._compat import exact_div as _exact_div

    assert idxs_ap.dtype == mybir.dt.int16
    assert in_ap.dtype == out_ap.dtype
    stride_bytes = elem_step * mybir.dt.size(in_ap.dtype)
    stride_bytes_256 = _exact_div(stride_bytes, 256)
    _in_ap = gp.lower_ap_dma(in_ap, for_custom_bir_dma=True)
    _idxs_ap = gp.lower_ap(idxs_ap)
    _out_ap = gp.lower_ap(out_ap)
    return gp.add_instruction(
        mybir.InstDMAGatherAnt(
            name=gp.bass.get_next_instruction_name(),
            ins=[*_in_ap, _idxs_ap, gp.lower_val_access(gp.to_reg(num_idxs_reg))],
            outs=[_out_ap],
            transpose=False,
            num_idxs=num_idxs,
            elem_size=elem_size,
            stride_bytes_256=stride_bytes_256,
            gen_mode=0,
            single_packet=True,
            queue_num=queue_num,
            sbuf_tokens_per_rank=0,
            sbuf_free_dim_per_rank=0,
            sbuf_free_dim_pad_per_rank=0,
            sbuf_byte_offset=0,
        )
    )


def _build_program(NB, KTOT, chunk_bl, chunk_c, chunk_first, chunk_last, IN_CH, HID, OUT_CH):
    """SPMD program; per-core data comes via input tensors.

    Cells swept in (bl, dst-core) order; chunk_bl/chunk_c give each
    chunk's dst block (local id) and dst core; first/last flag cell
    boundaries (identical across cores)."""
    NPC = NB * P
    TB = NB * NCORES
    KT = IN_CH // P
    assert IN_CH % P == 0 and HID == P and OUT_CH * 2 == P
    IDXC = KTOT * 8  # idx table columns ( = KTOT*128/16 )
    ROWB = 256  # fp8 table row stride in bytes (gather stride granularity)

    nc = bass.Bass(num_swdge_queues=4, dynamic_dma_scratch_size=16 * GC * P)
    xT2 = nc.dram_tensor("xT2", [P, KT * NPC], BF16, kind="ExternalInput")
    W1b = nc.dram_tensor("W1b", [P, KT * HID], BF16, kind="ExternalInput")
    W2b = nc.dram_tensor("W2b", [P, OUT_CH], BF16, kind="ExternalInput")
    disT = nc.dram_tensor("disT", [P, NB], F32, kind="ExternalInput")  # s1*dis
    dissqT = nc.dram_tensor("dissqT", [P, NB], F32, kind="ExternalInput")  # s2*dis^2
    disO = nc.dram_tensor("disO", [P, NB], F32, kind="ExternalInput")  # dis/s2
    idx1 = nc.dram_tensor("idx1", [P, IDXC], I16, kind="ExternalInput")
    dstlA = nc.dram_tensor("dstlA", [P, KTOT], F32, kind="ExternalInput")
    outY = nc.dram_tensor("outY", [P, NB * OUT_CH], F32, kind="ExternalOutput")

    h1s = nc.dram_tensor("h1s", [NPC, ROWB], FP8)  # row = [h1p, pad] (256B)
    h2s = nc.dram_tensor("h2s", [NPC, ROWB], FP8)  # row = [h2p, pad] (256B)
    # partial/agg tables split by bl range so each ReduceScatter piece is a
    # contiguous tensor (walrus requires contiguous collective operands)
    RS_BOUNDS = [0, 32, 64, 88, NB]  # piece k covers bl [RS_BOUNDS[k], RS_BOUNDS[k+1])
    NPIECE = len(RS_BOUNDS) - 1
    PIECE_NB = [RS_BOUNDS[k + 1] - RS_BOUNDS[k] for k in range(NPIECE)]
    part1 = [
        nc.dram_tensor(f"part1_{k}", [NCORES, P * PIECE_NB[k] * HID], BF16)
        for k in range(NPIECE)
    ]
    part2 = [
        nc.dram_tensor(f"part2_{k}", [NCORES, P * PIECE_NB[k] * OUT_CH], BF16)
        for k in range(NPIECE)
    ]
    agg1 = [
        nc.dram_tensor(f"agg1_{k}", [P, PIECE_NB[k] * HID], BF16)
        for k in range(NPIECE)
    ]
    agg2 = [
        nc.dram_tensor(f"agg2_{k}", [P, PIECE_NB[k] * OUT_CH], BF16)
        for k in range(NPIECE)
    ]

    def piece_of(bl):
        for k in range(NPIECE):
            if bl < RS_BOUNDS[k + 1]:
                return k
        raise AssertionError(bl)

    rg = [list(range(NCORES))]
    RELU = mybir.ActivationFunctionType.Relu
    ADD = mybir.AluOpType.add
    ISEQ = mybir.AluOpType.is_equal

    cnt_regs = {}  # reused num_idxs registers (Pool regfile is tiny)

    def cnt_reg(n):
        if n not in cnt_regs:
            cnt_regs[n] = nc.gpsimd.to_reg(n)
        return cnt_regs[n]

    # per-chunk psum grouping: cells grouped by gb//GRP into one psum bank
    # pair; GRP is a multiple of NCORES so groups span whole dst blocks.
    GRP1 = 8  # cells per psum group in layer 1 (8*128 = 1024 f32 = 2 banks)
    GRP2 = 16  # cells per psum group in layer 2 (16*64 = 1024 f32 = 2 banks)
    SLAB_B = 4  # dst blocks per staging slab (x8 cores = 32 cells)
    EPI_DELAY = 12  # blocks between an RS firing and its epilogue being issued

    with tile.TileContext(nc) as tc:
        with tc.tile_pool(name="const", bufs=1) as cst:
            w1sb = cst.tile([P, KT * HID], BF16)
            nc.sync.dma_start(out=w1sb[:], in_=W1b[:, :])
            w2sb = cst.tile([P, OUT_CH], BF16)
            nc.sync.dma_start(out=w2sb[:], in_=W2b[:, :])
            dissb = cst.tile([P, NB], F32)
            nc.sync.dma_start(out=dissb[:], in_=disT[:, :])
            dissqsb = cst.tile([P, NB], F32)
            nc.sync.dma_start(out=dissqsb[:], in_=dissqT[:, :])
            disosb = cst.tile([P, NB], F32)
            nc.sync.dma_start(out=disosb[:], in_=disO[:, :])
            idx1sb = cst.tile([P, IDXC], I16)
            nc.sync.dma_start(out=idx1sb[:], in_=idx1[:, :])
            dstlAsb = cst.tile([P, KTOT], F32)
            nc.sync.dma_start(out=dstlAsb[:], in_=dstlA[:, :])
            iotasb = cst.tile([P, P], BF16)
            nc.gpsimd.iota(
                iotasb[:],
                pattern=[[1, P]],
                base=0,
                channel_multiplier=0,
                allow_small_or_imprecise_dtypes=True,
            )
            idsb = cst.tile([P, P], BF16)
            make_identity(nc, idsb[:])
            nc.gpsimd.load_library(mlp)

            # scaled-by-S1 self values (bf16) + fp8 table staging
            h1stage = cst.tile([P, NB * HID], BF16)
            h1tab = cst.tile([P, NB * HID], FP8)
            h2stage = cst.tile([P, NB * OUT_CH], BF16)
            h2tab = cst.tile([P, NB * OUT_CH], FP8)

            # ---- Phase A: h1' = (x @ W1) * (s1*dis)
            with (
                tc.tile_pool(name="pa", bufs=3) as pa,
                tc.tile_pool(name="pap", bufs=2, space="PSUM") as pap,
            ):
                XB = 4
                for q in range(0, NB, XB):
                    nb = min(XB, NB - q)
                    xt = pa.tile([P, KT * XB * P], BF16, tag="xt")
                    for k in range(KT):
                        nc.sync.dma_start(
                            out=xt[:, k * XB * P : k * XB * P + nb * P],
                            in_=xT2[:, k * NPC + q * P : k * NPC + (q + nb) * P],
                        )
                    for j in range(nb):
                        b = q + j
                        ps = pap.tile([P, HID], F32, tag="ps")
                        for k in range(KT):
                            nc.tensor.matmul(
                                ps[:],
                                lhsT=xt[:, k * XB * P + j * P : k * XB * P + (j + 1) * P],
                                rhs=w1sb[:, k * HID : (k + 1) * HID],
                                start=(k == 0),
                                stop=(k == KT - 1),
                            )
                        nc.vector.tensor_scalar_mul(
                            h1stage[:, b * HID : (b + 1) * HID],
                            ps[:],
                            dissb[:, b : b + 1],
                        )
                        nc.vector.tensor_scalar_mul(
                            h1tab[:, b * HID : (b + 1) * HID],
                            ps[:],
                            dissb[:, b : b + 1],
                        )
            nc.sync.dma_start(
                out=h1s[:, :].rearrange("(p b) f -> p b f", p=P)[:, :, 0:HID],
                in_=h1tab[:].rearrange("p (b f) -> p b f", f=HID),
            )

            # ---- gather + one-hot reduce + piecewise RS, shared by layers
            RS_SPLITS = RS_BOUNDS[1:]  # bl boundaries at which RS pieces fire

            def agg_layer(table_ap, elem, F, GRP, part, agg, piece_epi):
                """Gather + one-hot reduce + partial writes, sweeping cells in
                (bl, dst-core) order; fires a ReduceScatter piece as the sweep
                passes each bl in RS_SPLITS.  piece_epi(k) is invoked (late,
                EPI_DELAY blocks after piece k's RS fired) so downstream
                epilogue work interleaves into the sweep without queue stalls."""
                rs_done = 0
                epi_done = 0
                assert GRP % NCORES == 0

                def part_view(bl):
                    # (tensor, local bl offset) for a given global bl
                    k = piece_of(bl)
                    return part[k], RS_BOUNDS[k], PIECE_NB[k]

                def emit_rs(upto_bl):
                    nonlocal rs_done
                    while rs_done < len(RS_SPLITS) and upto_bl >= RS_SPLITS[rs_done]:
                        nc.gpsimd.collective_compute(
                            "ReduceScatter",
                            ADD,
                            replica_groups=rg,
                            ins=[part[rs_done][:, :]],
                            outs=[agg[rs_done][:, :]],
                        )
                        rs_done += 1

                def emit_epi(upto_bl):
                    nonlocal epi_done
                    while epi_done < rs_done and (
                        upto_bl >= RS_SPLITS[epi_done] + EPI_DELAY
                        or upto_bl >= NB + 10_000
                    ):
                        piece_epi(epi_done)
                        epi_done += 1

                with (
                    tc.tile_pool(name="pg", bufs=4) as pg,
                    tc.tile_pool(name="pm", bufs=8) as pm,
                    tc.tile_pool(name="pst", bufs=2) as pstp,
                    tc.tile_pool(name="pgp", bufs=2, space="PSUM") as pgp,
                ):
                    pgrp = None
                    stag = None
                    slab0 = 0
                    for gq, gi in enumerate(range(0, KTOT, GC)):
                        nchunk = min(GC, KTOT - gi)
                        g = pg.tile([P, GC * elem], FP8, tag="g")
                        _dma_gather_raw(
                            nc.gpsimd,
                            g[:, : nchunk * elem].rearrange(
                                "p (c f) -> p c f", f=elem
                            ),
                            table_ap,
                            idx1sb[:, gi * 8 : (gi + nchunk) * 8],
                            nchunk * P,
                            cnt_reg(nchunk * P),
                            elem,
                            ROWB,  # fp8: 256 elements = 256B row stride
                            queue_num=gq % 4,
                        )
                        for j in range(nchunk):
                            col = gi + j
                            bl = chunk_bl[col]
                            cc = chunk_c[col]
                            sw = bl * NCORES + cc
                            gsl = sw % GRP
                            if chunk_first[col] and gsl == 0:
                                pgrp = pgp.tile([P, GRP * F], F32, tag="pgrp")
                            if chunk_first[col] and sw % (NCORES * SLAB_B) == 0:
                                stag = pstp.tile(
                                    [P, SLAB_B * NCORES * F], BF16, tag="stag"
                                )
                                slab0 = bl
                            m = pm.tile([P, P], BF16, tag="m")
                            nc.vector.tensor_scalar(
                                m[:],
                                iotasb[:, :P],
                                dstlAsb[:, col : col + 1],
                                None,
                                op0=ISEQ,
                            )
                            nc.tensor.matmul(
                                pgrp[:, gsl * F : (gsl + 1) * F],
                                lhsT=m[:],
                                rhs=g[:, j * elem : j * elem + F],
                                start=chunk_first[col],
                                stop=chunk_last[col],
                            )
                            last_cell = sw == TB - 1
                            if chunk_last[col] and (gsl == GRP - 1 or last_cell):
                                # copy finished psum group -> staging (c-major)
                                nbsp = gsl // NCORES + 1  # blocks in this group
                                b0 = (bl - slab0) - (nbsp - 1)
                                nc.scalar.copy(
                                    out=stag.rearrange(
                                        "p (c b f) -> p c b f",
                                        c=NCORES,
                                        b=SLAB_B,
                                    )[:, :, b0 : b0 + nbsp, :],
                                    in_=pgrp[:, : (gsl + 1) * F].rearrange(
                                        "p (b c f) -> p c b f", c=NCORES, f=F
                                    ),
                                )
                            if chunk_last[col] and (
                                sw % (NCORES * SLAB_B) == NCORES * SLAB_B - 1
                                or last_cell
                            ):
                                nbl = bl - slab0 + 1
                                stv = stag.rearrange(
                                    "p (c b f) -> p c b f", c=NCORES, b=SLAB_B
                                )
                                pt, plo, pnb = part_view(slab0)
                                nc.sync.dma_start(
                                    out=pt[:, :].rearrange(
                                        "c (p b f) -> p c b f", p=P, f=F
                                    )[:, :, slab0 - plo : slab0 - plo + nbl, :],
                                    in_=stv[:, :, 0:nbl, :],
                                )
                                emit_rs(bl + 1)
                                emit_epi(bl + 1)
                    emit_rs(NB + 20_000)
                    emit_epi(NB + 20_000)

            # ---- Epilogue 1 (per RS piece) + dense layer 2 + h2 table write
            pe1s = tc.alloc_tile_pool(name="pe1s", bufs=2)
            pe1 = tc.alloc_tile_pool(name="pe1", bufs=8)
            pe1p = tc.alloc_tile_pool(name="pe1p", bufs=2, space="PSUM")
            pe1q = tc.alloc_tile_pool(name="pe1q", bufs=2, space="PSUM")
            AGB = 8

            def epi1_slab(q, nb):
                k = piece_of(q)
                lo = q - RS_BOUNDS[k]
                asrc = agg1[k][:, lo * HID : (lo + nb) * HID]
                asb = pe1s.tile([P, AGB * HID], BF16, tag="asb")
                nc.sync.dma_start(out=asb[:, : nb * HID], in_=asrc)
                t0 = pe1s.tile([P, AGB * HID], BF16, tag="t0")
                nc.vector.tensor_tensor(
                    t0[:, : nb * HID],
                    asb[:, : nb * HID],
                    h1stage[:, q * HID : (q + nb) * HID],
                    op=ADD,
                )
                # relu(dis*(agg+h')) = dis*relu(agg+h'); the s1 scale on t0 is
                # cancelled by W2b = W2/s1, dis^2*s2 folded into dissqT
                r1 = pe1s.tile([P, AGB * HID], BF16, tag="r1")
                nc.scalar.activation(r1[:, : nb * HID], t0[:, : nb * HID], RELU)
                for j in range(nb):
                    b = q + j
                    pst = pe1p.tile([P, HID], BF16, tag="pst")
                    nc.tensor.transpose(
                        out=pst[:],
                        in_=r1[:, j * HID : (j + 1) * HID],
                        identity=idsb[:],
                    )
                    o1t = pe1.tile([P, HID], BF16, tag="o1t")
                    nc.scalar.copy(out=o1t[:], in_=pst[:])
                    ps2 = pe1q.tile([P, OUT_CH], F32, tag="ps2")
                    nc.tensor.matmul(
                        ps2[:], lhsT=o1t[:], rhs=w2sb[:], start=True, stop=True
                    )
                    nc.vector.tensor_scalar_mul(
                        h2stage[:, b * OUT_CH : (b + 1) * OUT_CH],
                        ps2[:],
                        dissqsb[:, b : b + 1],
                    )
                    nc.vector.tensor_scalar_mul(
                        h2tab[:, b * OUT_CH : (b + 1) * OUT_CH],
                        ps2[:],
                        dissqsb[:, b : b + 1],
                    )

            def epi1_piece(k):
                for q in range(RS_BOUNDS[k], RS_BOUNDS[k + 1], AGB):
                    epi1_slab(q, min(AGB, RS_BOUNDS[k + 1] - q))
                lo, hi = RS_BOUNDS[k], RS_BOUNDS[k + 1]
                nc.sync.dma_start(
                    out=h2s[:, :].rearrange("(p b) f -> p b f", p=P)[
                        :, lo:hi, 0:OUT_CH
                    ],
                    in_=h2tab[:, lo * OUT_CH : hi * OUT_CH].rearrange(
                        "p (b f) -> p b f", f=OUT_CH
                    ),
                )

            def epi2_piece(k):
                for q in range(RS_BOUNDS[k], RS_BOUNDS[k + 1], AGB):
                    nb = min(AGB, RS_BOUNDS[k + 1] - q)
                    lo = q - RS_BOUNDS[k]
                    a2src = agg2[k][:, lo * OUT_CH : (lo + nb) * OUT_CH]
                    asb2 = pe2s.tile([P, AGB * OUT_CH], BF16, tag="asb2")
                    nc.sync.dma_start(out=asb2[:, : nb * OUT_CH], in_=a2src)
                    t2 = pe2s.tile([P, AGB * OUT_CH], BF16, tag="t2")
                    nc.vector.tensor_tensor(
                        t2[:, : nb * OUT_CH],
                        asb2[:, : nb * OUT_CH],
                        h2stage[:, q * OUT_CH : (q + nb) * OUT_CH],
                        op=ADD,
                    )
                    r2 = pe2s.tile([P, AGB * OUT_CH], F32, tag="r2")
                    nc.scalar.activation(
                        r2[:, : nb * OUT_CH], t2[:, : nb * OUT_CH], RELU
                    )
                    ob = pe2s.tile([P, AGB * OUT_CH], F32, tag="ob")
                    for j in range(nb):
                        b = q + j
                        nc.vector.tensor_scalar_mul(
                            ob[:, j * OUT_CH : (j + 1) * OUT_CH],
                            r2[:, j * OUT_CH : (j + 1) * OUT_CH],
                            disosb[:, b : b + 1],
                        )
                    nc.sync.dma_start(
                        out=outY[:, q * OUT_CH : (q + nb) * OUT_CH],
                        in_=ob[:, : nb * OUT_CH],
                    )

            # ---- run the two sweeps (epilogue pools scoped per layer so
            # PSUM/SBUF budgets hold: L1 uses pgp(4)+pe1p(2)+pe1q(2) banks)
            agg_layer(h1s[:, 0:HID], HID, HID, GRP1, part1, agg1, epi1_piece)
            pe1q.release()
            pe1p.release()
            pe1.release()
            pe1s.release()

            pe2s = tc.alloc_tile_pool(name="pe2s", bufs=2)
            agg_layer(
                h2s[:, 0:OUT_CH], OUT_CH, OUT_CH, GRP2, part2, agg2, epi2_piece
            )
            pe2s.release()

    _spill_waits(nc)
    from concourse.library_overlay import lower_extended_insts

    lower_extended_insts(nc)
    return nc


def _prepare(x, src, dst):
    """Host-side: balanced block assignment + edge slot tables."""
    N = x.shape[0]
    assert N % NCORES == 0
    NPR = N // NCORES
    # blocks per core: keep (core, block) cells under ~245 edges on average
    E = src.shape[0]
    NB = max(
        int(math.ceil(NPR / P)),
        int(math.ceil(E / (NCORES * NCORES * P * 1.9))),
    )
    TB = NB * NCORES
    NPC = NB * P

    indeg = np.bincount(dst, minlength=N).astype(np.int64)
    dis = (1.0 / np.sqrt(indeg.astype(np.float32) + 1.0)).astype(np.float32)

    cs = (src // NPR).astype(np.int64)
    np.clip(cs, 0, NCORES - 1, out=cs)
    # v[d, c] = in-edges of d from core c
    v = np.bincount(dst * NCORES + cs, minlength=N * NCORES).reshape(N, NCORES)
    v = v.astype(np.int32)

    bl_of = np.empty(N, np.int32)  # block (within home core)
    lane_of = np.empty(N, np.int32)
    for k in range(NCORES):
        nodes = np.arange(k * NPR, (k + 1) * NPR)
        order = nodes[np.argsort(-indeg[nodes], kind="stable")]
        loads = np.zeros((NB, NCORES), np.int32)
        counts = np.zeros(NB, np.int32)
        full = np.zeros(NB, bool)
        for node in order:
            cand = (loads + v[node]).max(axis=1) + counts * 1e-4
            cand[full] = np.inf
            b = int(np.argmin(cand))
            bl_of[node] = b
            lane_of[node] = counts[b]
            loads[b] += v[node]
            counts[b] += 1
            if counts[b] >= P:
                full[b] = True

    home = np.arange(N) // NPR
    trow_of = lane_of * NB + bl_of  # local table row (lane-major)

    # per-(core, sweep-cell) counts -> shared chunk structure.
    # Sweep order is (bl, dst core): partials for low bl complete first on
    # every core, letting ReduceScatter pieces overlap the rest of the sweep.
    esw = bl_of[dst] * NCORES + home[dst]
    cellcnt = np.bincount(cs * TB + esw, minlength=NCORES * TB).reshape(NCORES, TB)
    k_b = np.maximum(1, -(-cellcnt.max(axis=0) // P))  # ceil(max/128), >=1
    chunk_start = np.zeros(TB + 1, np.int64)
    np.cumsum(k_b, out=chunk_start[1:])
    KTOT = int(chunk_start[-1])

    chunk_sw = np.repeat(np.arange(TB), k_b)
    idx_in_cell = np.arange(KTOT) - chunk_start[chunk_sw]
    chunk_first = idx_in_cell == 0
    chunk_last = idx_in_cell == (k_b[chunk_sw] - 1)

    # per-core slot tables
    esrc = np.zeros((NCORES, P, KTOT), np.int32)
    dstlA = np.full((NCORES, P, KTOT), PAD_LANE, np.float32)
    for k in range(NCORES):
        sel = np.where(cs == k)[0]
        eord = sel[np.argsort(esw[sel], kind="stable")]
        e_sw = esw[eord]
        # position within cell
        cnt = cellcnt[k]
        ofs = np.zeros(TB + 1, np.int64)
        np.cumsum(cnt, out=ofs[1:])
        pos = np.arange(eord.size) - ofs[e_sw]
        chunk = chunk_start[e_sw] + pos // P
        lane = pos % P
        tr = trow_of[src[eord]]
        esrc[k, lane, chunk] = tr
        dstlA[k, lane, chunk] = (lane_of[dst[eord]]).astype(np.float32)

    return dict(
        NB=NB, TB=TB, NPC=NPC, KTOT=KTOT, dis=dis,
        bl_of=bl_of, lane_of=lane_of, trow_of=trow_of,
        chunk_bl=(chunk_sw // NCORES).tolist(),
        chunk_c=(chunk_sw % NCORES).tolist(),
        chunk_first=chunk_first.tolist(),
        chunk_last=chunk_last.tolist(),
        esrc=esrc, dstlA=dstlA,
    )


def kernel(x, edge_index, W1, b1, W2, b2):
    x = np.ascontiguousarray(np.asarray(x, dtype=np.float32))
    W1 = np.asarray(W1, dtype=np.float32)
    W2 = np.asarray(W2, dtype=np.float32)
    b1 = np.asarray(b1, dtype=np.float32)
    b2 = np.asarray(b2, dtype=np.float32)
    assert not np.any(b1) and not np.any(b2), "zero-bias fast path only"
    src = np.asarray(edge_index[0]).astype(np.int64)
    dst = np.asarray(edge_index[1]).astype(np.int64)

    N, IN_CH = x.shape
    HID = W1.shape[1]
    OUT_CH = W2.shape[1]
    KT = IN_CH // P
    pr = _prepare(x, src, dst)
    NB, NPC, KTOT = pr["NB"], pr["NPC"], pr["KTOT"]
    NPR = N // NCORES

    nc = _build_program(
        NB, KTOT, pr["chunk_bl"], pr["chunk_c"], pr["chunk_first"],
        pr["chunk_last"], IN_CH, HID, OUT_CH,
    )

    W1b = np.zeros((P, KT * HID), np.float32)
    for k in range(KT):
        W1b[:, k * HID : (k + 1) * HID] = W1[k * P : (k + 1) * P, :]

    bl_of, lane_of = pr["bl_of"], pr["lane_of"]
    in_maps = []
    for c in range(NCORES):
        nodes = np.arange(c * NPR, (c + 1) * NPR)
        slot = bl_of[nodes] * P + lane_of[nodes]  # block-major slot in [0,NPC)
        xs = np.zeros((NPC, IN_CH), np.float32)
        xs[slot] = x[nodes]
        xT2 = np.zeros((P, KT * NPC), np.float32)
        for k in range(KT):
            xT2[:, k * NPC : (k + 1) * NPC] = xs[:, k * P : (k + 1) * P].T
        disc = np.ones((P, NB), np.float32)
        disc[lane_of[nodes], bl_of[nodes]] = pr["dis"][nodes]

        # idx1: wrapped-16 int16 per GC-chunk gather instruction
        esrc_c = pr["esrc"][c]  # [P, KTOT] lane, chunk
        flat = esrc_c.T.reshape(-1)  # [(chunk, lane)]
        idx1 = np.zeros((16, KTOT * 8), np.int16)
        for gi in range(0, KTOT, GC):
            nchunk = min(GC, KTOT - gi)
            seg = flat[gi * P : (gi + nchunk) * P]
            i = np.arange(seg.size)
            blockcols = np.zeros((16, nchunk * 8), np.int16)
            blockcols[i % 16, i // 16] = seg.astype(np.int16)
            idx1[:, gi * 8 : (gi + nchunk) * 8] = blockcols
        idx1 = np.tile(idx1, (8, 1))  # replicate across the 8 Q7 cores

        dstlA = pr["dstlA"][c]

        in_maps.append(
            {
                "xT2": xT2.astype(ml_dtypes.bfloat16),
                "W1b": W1b.astype(ml_dtypes.bfloat16),
                "W2b": (W2 / S1).astype(ml_dtypes.bfloat16),
                "disT": disc * S1,
                "dissqT": disc * disc * S2,
                "disO": disc / S2,
                "idx1": idx1,
                "dstlA": dstlA.astype(np.float32),
            }
        )

    res = run_bass_kernel_spmd(nc, in_maps, core_ids=list(range(NCORES)))
    global _last_results, _last_nc
    _last_results = res
    _last_nc = nc

    out = np.empty((N, OUT_CH), np.float32)
    for c in range(NCORES):
        oc = res.results[c]["outY"]  # [P, NB*OUT_CH]
        nodes = np.arange(c * NPR, (c + 1) * NPR)
        out[nodes] = oc[
            lane_of[nodes][:, None],
            (bl_of[nodes][:, None] * OUT_CH + np.arange(OUT_CH)[None, :]),
        ]
    return out


# revision 14
# speedup vs baseline: 1.0524x; 1.0524x over previous
"""2-layer GCN on 8 trn2 cores — src-partitioned with ReduceScatter.

Per layer (h' = (x@W)*dis, dis = (deg+1)^-1/2):
  1. Each core computes h' for its OWN 12500 nodes (x@W on PE), writes the
     local table (fp8-e3m4, scaled by a power of 2 to sit in e3m4's normal
     range; rows padded to 256B for the gather stride) to DRAM.
  2. Each core gathers h'[src] rows for its own ~200k out-edges (sorted by
     global dst block) with big merged dma_gather ops, and reduces them into
     per-dst-block partial sums via one-hot matmuls accumulated in PSUM.
     One-hot masks are built per chunk with tensor_scalar is_equal (runs in
     the DVE 4x_2p fast mode).
  3. Partial tables (bf16) are summed across cores with piecewise
     ReduceScatters that overlap the sweep; per-piece epilogues are issued
     late (after the RS landed) so they interleave with the sweep without
     stalling engine queues.
  4. Epilogue: out = relu(dis*(agg + h'_self)); fp8 scale factors are folded
     into host-provided dis vectors and W2, so no extra unscale ops exist.

Edges live on the core that owns their SOURCE node, so the gather reads are
all core-local (int16 row ids fit) and the only collectives are the
(cheap, output-sized) ReduceScatters.  Host-side, nodes are packed into
blocks with an 8-dim greedy balancer so each (core, dst-block) cell fits in
2 chunks of 128 edge slots.
"""

import math

import ml_dtypes
import numpy as np

import concourse.bass as bass
import concourse.mybir as mybir
import concourse.tile as tile
from concourse.bass_utils import run_bass_kernel_spmd
from concourse.library_config import mlp
from concourse.masks import make_identity
from concourse.vector_clock import ScopedClock

P = 128
NCORES = 8
F32 = mybir.dt.float32
BF16 = mybir.dt.bfloat16
FP8 = mybir.dt.float8e3  # e3m4
I32 = mybir.dt.int32
I16 = mybir.dt.int16
PAD_LANE = 1000.0
GC = 8  # chunks per dma_gather instruction (1024 idx = SWDGE cap)
S1 = 4.0  # layer-1 fp8 table scale (folded into disT and W2)
S2 = 16.0  # layer-2 fp8 table scale (folded into dissqT and disO)


def _patched_drain_and_barrier(self, tick_clock, wait_clock):
    # This walrus build rejects >1 sem wait on TPB_CTRL (Drain) instructions.
    drain_inst = self.nc.sync.drain()
    wait_clock.add_sem_waits(
        drain_inst.ins, ScopedClock({None: tick_clock.global_clock})
    )
    si = drain_inst.ins.sync_info
    waits = list(si.on_wait)
    if len(waits) > 1:
        while len(si.on_wait):
            si.on_wait.pop()
        si.on_wait.append(waits[0])
        for w in waits[1:]:
            d2 = self.nc.sync.drain(fusable=False)
            si2 = d2.ins.sync_info
            if si2 is None:
                d2.ins.sync_info = mybir.SyncInfo(on_wait=[w], on_update=[])
            else:
                si2.on_wait.append(w)
    self.nc.all_engine_barrier()
    popped = self.nc._tile_sem_poison_stack.pop()
    assert popped is self._sem_poison
    self.nc.clear_and_free_semaphores(list(self.sems.allocated().values()))
    self.nc.all_engine_barrier()


tile.TileContext._drain_and_barrier = _patched_drain_and_barrier


def _spill_waits(nc, max_waits=1):
    """Move extra sync waits onto dedicated single-wait NoOps (walrus limit)."""
    n = 0
    for f in nc.m.functions:
        for blk in f.blocks:
            il = blk.instructions
            out = []
            for inst in il:
                si = inst.sync_info
                if si is not None and len(si.on_wait) > max_waits:
                    waits = list(si.on_wait)
                    while len(si.on_wait):
                        si.on_wait.pop()
                    for w in waits[:max_waits]:
                        si.on_wait.append(w)
                    for w in waits[max_waits:]:
                        nop = mybir.InstNoOp(
                            name=f"waitspill-{n}",
                            sync_info=mybir.SyncInfo(on_wait=[w], on_update=[]),
                            bass_nofuse=True,
                            engine=inst.engine,
                        )
                        n += 1
                        out.append(nop)
                out.append(inst)
            blk.instructions = out
    return n


def _dma_gather_raw(
    gp, out_ap, in_ap, idxs_ap, num_idxs, num_idxs_reg, elem_size, elem_step,
    queue_num=0,
):
    """dma_gather (non-transpose, DRAM source) without the elem%256B assert:
    elem_step (row stride) must be a 256B multiple, elem_size may be a
    fraction of a row.  Descriptors move elem_size*dtype bytes each."""
    from concourse._compat import exact_div as _exact_div

    assert idxs_ap.dtype == mybir.dt.int16
    assert in_ap.dtype == out_ap.dtype
    stride_bytes = elem_step * mybir.dt.size(in_ap.dtype)
    stride_bytes_256 = _exact_div(stride_bytes, 256)
    _in_ap = gp.lower_ap_dma(in_ap, for_custom_bir_dma=True)
    _idxs_ap = gp.lower_ap(idxs_ap)
    _out_ap = gp.lower_ap(out_ap)
    return gp.add_instruction(
        mybir.InstDMAGatherAnt(
            name=gp.bass.get_next_instruction_name(),
            ins=[*_in_ap, _idxs_ap, gp.lower_val_access(gp.to_reg(num_idxs_reg))],
            outs=[_out_ap],
            transpose=False,
            num_idxs=num_idxs,
            elem_size=elem_size,
            stride_bytes_256=stride_bytes_256,
            gen_mode=0,
            single_packet=True,
            queue_num=queue_num,
            sbuf_tokens_per_rank=0,
            sbuf_free_dim_per_rank=0,
            sbuf_free_dim_pad_per_rank=0,
            sbuf_byte_offset=0,
        )
    )


def _build_program(NB, KTOT, chunk_bl, chunk_c, chunk_first, chunk_last, IN_CH, HID, OUT_CH):
    """SPMD program; per-core data comes via input tensors.

    Cells swept in (bl, dst-core) order; chunk_bl/chunk_c give each
    chunk's dst block (local id) and dst core; first/last flag cell
    boundaries (identical across cores)."""
    NPC = NB * P
    TB = NB * NCORES
    KT = IN_CH // P
    assert IN_CH % P == 0 and HID == P and OUT_CH * 2 == P
    IDXC = KTOT * 8  # idx table columns ( = KTOT*128/16 )
    ROWB = 256  # fp8 table row stride in bytes (gather stride granularity)

    nc = bass.Bass(num_swdge_queues=4)
    xT2 = nc.dram_tensor("xT2", [P, KT * NPC], BF16, kind="ExternalInput")
    W1b = nc.dram_tensor("W1b", [P, KT * HID], BF16, kind="ExternalInput")
    W2b = nc.dram_tensor("W2b", [P, OUT_CH], BF16, kind="ExternalInput")
    disT = nc.dram_tensor("disT", [P, NB], F32, kind="ExternalInput")  # s1*dis
    dissqT = nc.dram_tensor("dissqT", [P, NB], F32, kind="ExternalInput")  # s2*dis^2
    disO = nc.dram_tensor("disO", [P, NB], F32, kind="ExternalInput")  # dis/s2
    idx1 = nc.dram_tensor("idx1", [P, IDXC], I16, kind="ExternalInput")
    dstlA = nc.dram_tensor("dstlA", [P, KTOT], F32, kind="ExternalInput")
    outY = nc.dram_tensor("outY", [P, NB * OUT_CH], F32, kind="ExternalOutput")

    h1s = nc.dram_tensor("h1s", [NPC, ROWB], FP8)  # fp8 rows padded to 256B
    h2s = nc.dram_tensor("h2s", [NPC, ROWB], FP8)
    # partial/agg tables split by bl range so each ReduceScatter piece is a
    # contiguous tensor (walrus requires contiguous collective operands)
    RS_BOUNDS = [0, 32, 64, 88, NB]  # piece k covers bl [RS_BOUNDS[k], RS_BOUNDS[k+1])
    NPIECE = len(RS_BOUNDS) - 1
    PIECE_NB = [RS_BOUNDS[k + 1] - RS_BOUNDS[k] for k in range(NPIECE)]
    part1 = [
        nc.dram_tensor(f"part1_{k}", [NCORES, P * PIECE_NB[k] * HID], BF16)
        for k in range(NPIECE)
    ]
    part2 = [
        nc.dram_tensor(f"part2_{k}", [NCORES, P * PIECE_NB[k] * OUT_CH], BF16)
        for k in range(NPIECE)
    ]
    agg1 = [
        nc.dram_tensor(f"agg1_{k}", [P, PIECE_NB[k] * HID], BF16)
        for k in range(NPIECE)
    ]
    agg2 = [
        nc.dram_tensor(f"agg2_{k}", [P, PIECE_NB[k] * OUT_CH], BF16)
        for k in range(NPIECE)
    ]

    def piece_of(bl):
        for k in range(NPIECE):
            if bl < RS_BOUNDS[k + 1]:
                return k
        raise AssertionError(bl)

    rg = [list(range(NCORES))]
    RELU = mybir.ActivationFunctionType.Relu
    ADD = mybir.AluOpType.add
    ISEQ = mybir.AluOpType.is_equal

    cnt_regs = {}  # reused num_idxs registers (Pool regfile is tiny)

    def cnt_reg(n):
        if n not in cnt_regs:
            cnt_regs[n] = nc.gpsimd.to_reg(n)
        return cnt_regs[n]

    # per-chunk psum grouping: cells grouped by gb//GRP into one psum bank
    # pair; GRP is a multiple of NCORES so groups span whole dst blocks.
    GRP1 = 8  # cells per psum group in layer 1 (8*128 = 1024 f32 = 2 banks)
    GRP2 = 16  # cells per psum group in layer 2 (16*64 = 1024 f32 = 2 banks)
    SLAB_B = 4  # dst blocks per staging slab (x8 cores = 32 cells)
    EPI_DELAY = 20  # blocks between an RS firing and its epilogue being issued

    with tile.TileContext(nc) as tc:
        with tc.tile_pool(name="const", bufs=1) as cst:
            w1sb = cst.tile([P, KT * HID], BF16)
            nc.sync.dma_start(out=w1sb[:], in_=W1b[:, :])
            w2sb = cst.tile([P, OUT_CH], BF16)
            nc.sync.dma_start(out=w2sb[:], in_=W2b[:, :])
            dissb = cst.tile([P, NB], F32)
            nc.sync.dma_start(out=dissb[:], in_=disT[:, :])
            dissqsb = cst.tile([P, NB], F32)
            nc.sync.dma_start(out=dissqsb[:], in_=dissqT[:, :])
            disosb = cst.tile([P, NB], F32)
            nc.sync.dma_start(out=disosb[:], in_=disO[:, :])
            idx1sb = cst.tile([P, IDXC], I16)
            nc.sync.dma_start(out=idx1sb[:], in_=idx1[:, :])
            dstlAsb = cst.tile([P, KTOT], F32)
            nc.sync.dma_start(out=dstlAsb[:], in_=dstlA[:, :])
            iotasb = cst.tile([P, P], BF16)
            nc.gpsimd.iota(
                iotasb[:],
                pattern=[[1, P]],
                base=0,
                channel_multiplier=0,
                allow_small_or_imprecise_dtypes=True,
            )
            idsb = cst.tile([P, P], BF16)
            make_identity(nc, idsb[:])
            nc.gpsimd.load_library(mlp)

            # scaled-by-S1 self values (bf16) + fp8 table staging
            h1stage = cst.tile([P, NB * HID], BF16)
            h1tab = cst.tile([P, NB * HID], FP8)
            h2stage = cst.tile([P, NB * OUT_CH], BF16)
            h2tab = cst.tile([P, NB * OUT_CH], FP8)

            # ---- Phase A: h1' = (x @ W1) * (s1*dis)
            with (
                tc.tile_pool(name="pa", bufs=3) as pa,
                tc.tile_pool(name="pap", bufs=3, space="PSUM") as pap,
            ):
                XB = 4
                for q in range(0, NB, XB):
                    nb = min(XB, NB - q)
                    xt = pa.tile([P, KT * XB * P], BF16, tag="xt")
                    for k in range(KT):
                        nc.sync.dma_start(
                            out=xt[:, k * XB * P : k * XB * P + nb * P],
                            in_=xT2[:, k * NPC + q * P : k * NPC + (q + nb) * P],
                        )
                    for j in range(nb):
                        b = q + j
                        ps = pap.tile([P, HID], F32, tag="ps")
                        for k in range(KT):
                            nc.tensor.matmul(
                                ps[:],
                                lhsT=xt[:, k * XB * P + j * P : k * XB * P + (j + 1) * P],
                                rhs=w1sb[:, k * HID : (k + 1) * HID],
                                start=(k == 0),
                                stop=(k == KT - 1),
                            )
                        nc.scalar.mul(
                            h1stage[:, b * HID : (b + 1) * HID],
                            ps[:],
                            dissb[:, b : b + 1],
                        )
                        nc.vector.tensor_scalar_mul(
                            h1tab[:, b * HID : (b + 1) * HID],
                            ps[:],
                            dissb[:, b : b + 1],
                        )
            nc.sync.dma_start(
                out=h1s[:, :].rearrange("(p b) f -> p b f", p=P)[:, :, 0:HID],
                in_=h1tab[:].rearrange("p (b f) -> p b f", f=HID),
            )

            # ---- gather + one-hot reduce + piecewise RS, shared by layers
            RS_SPLITS = RS_BOUNDS[1:]  # bl boundaries at which RS pieces fire

            def agg_layer(table_ap, elem, F, GRP, part, agg, piece_epi):
                """Gather + one-hot reduce + partial writes, sweeping cells in
                (bl, dst-core) order; fires a ReduceScatter piece as the sweep
                passes each bl in RS_SPLITS.  piece_epi(k) is invoked (late,
                EPI_DELAY blocks after piece k's RS fired) so downstream
                epilogue work interleaves into the sweep without queue stalls."""
                rs_done = 0
                epi_done = 0
                assert GRP % NCORES == 0

                def part_view(bl):
                    # (tensor, local bl offset) for a given global bl
                    k = piece_of(bl)
                    return part[k], RS_BOUNDS[k], PIECE_NB[k]

                def emit_rs(upto_bl):
                    nonlocal rs_done
                    while rs_done < len(RS_SPLITS) and (
                        upto_bl >= RS_SPLITS[rs_done] + SLAB_B
                        or upto_bl > NB + 10_000
                    ):
                        nc.gpsimd.collective_compute(
                            "ReduceScatter",
                            ADD,
                            replica_groups=rg,
                            ins=[part[rs_done][:, :]],
                            outs=[agg[rs_done][:, :]],
                        )
                        rs_done += 1

                def emit_epi(upto_bl):
                    nonlocal epi_done
                    while epi_done < rs_done and (
                        upto_bl >= RS_SPLITS[epi_done] + EPI_DELAY
                        or upto_bl >= NB + 10_000
                    ):
                        piece_epi(epi_done)
                        epi_done += 1

                with (
                    tc.tile_pool(name="pg", bufs=4) as pg,
                    tc.tile_pool(name="pm", bufs=24) as pm,
                    tc.tile_pool(name="pst", bufs=2) as pstp,
                    tc.tile_pool(name="pgp", bufs=2, space="PSUM") as pgp,
                ):
                    pgrp = None
                    stag = None
                    slab0 = 0
                    for gq, gi in enumerate(range(0, KTOT, GC)):
                        nchunk = min(GC, KTOT - gi)
                        g = pg.tile([P, GC * elem], FP8, tag="g")
                        _dma_gather_raw(
                            nc.gpsimd,
                            g[:, : nchunk * elem].rearrange(
                                "p (c f) -> p c f", f=elem
                            ),
                            table_ap,
                            idx1sb[:, gi * 8 : (gi + nchunk) * 8],
                            nchunk * P,
                            cnt_reg(nchunk * P),
                            elem,
                            ROWB,  # fp8: 256 elements = 256B row stride
                            queue_num=gq % 4,
                        )
                        for j in range(nchunk):
                            col = gi + j
                            bl = chunk_bl[col]
                            cc = chunk_c[col]
                            sw = bl * NCORES + cc
                            gsl = sw % GRP
                            if chunk_first[col] and gsl == 0:
                                pgrp = pgp.tile([P, GRP * F], F32, tag="pgrp")
                            if chunk_first[col] and sw % (NCORES * SLAB_B) == 0:
                                stag = pstp.tile(
                                    [P, SLAB_B * NCORES * F], BF16, tag="stag"
                                )
                                slab0 = bl
                            m = pm.tile([P, P], BF16, tag="m")
                            nc.vector.tensor_scalar(
                                m[:],
                                iotasb[:, :P],
                                dstlAsb[:, col : col + 1],
                                None,
                                op0=ISEQ,
                            )
                            nc.tensor.matmul(
                                pgrp[:, gsl * F : (gsl + 1) * F],
                                lhsT=m[:],
                                rhs=g[:, j * elem : j * elem + F],
                                start=chunk_first[col],
                                stop=chunk_last[col],
                            )
                            last_cell = sw == TB - 1
                            if chunk_last[col] and (gsl == GRP - 1 or last_cell):
                                # copy finished psum group -> staging (c-major)
                                nbsp = gsl // NCORES + 1  # blocks in this group
                                b0 = (bl - slab0) - (nbsp - 1)
                                nc.scalar.copy(
                                    out=stag.rearrange(
                                        "p (c b f) -> p c b f",
                                        c=NCORES,
                                        b=SLAB_B,
                                    )[:, :, b0 : b0 + nbsp, :],
                                    in_=pgrp[:, : (gsl + 1) * F].rearrange(
                                        "p (b c f) -> p c b f", c=NCORES, f=F
                                    ),
                                )
                            if chunk_last[col] and (
                                sw % (NCORES * SLAB_B) == NCORES * SLAB_B - 1
                                or last_cell
                            ):
                                nbl = bl - slab0 + 1
                                stv = stag.rearrange(
                                    "p (c b f) -> p c b f", c=NCORES, b=SLAB_B
                                )
                                pt, plo, pnb = part_view(slab0)
                                nc.sync.dma_start(
                                    out=pt[:, :].rearrange(
                                        "c (p b f) -> p c b f", p=P, f=F
                                    )[:, :, slab0 - plo : slab0 - plo + nbl, :],
                                    in_=stv[:, :, 0:nbl, :],
                                )
                                emit_rs(bl + 1)
                                emit_epi(bl + 1)
                    emit_rs(NB + 20_000)
                    emit_epi(NB + 20_000)

            # ---- Epilogue 1 (per RS piece) + dense layer 2 + h2 table write
            pe1s = tc.alloc_tile_pool(name="pe1s", bufs=2)
            pe1 = tc.alloc_tile_pool(name="pe1", bufs=8)
            pe1p = tc.alloc_tile_pool(name="pe1p", bufs=2, space="PSUM")
            pe1q = tc.alloc_tile_pool(name="pe1q", bufs=2, space="PSUM")
            AGB = 8

            def epi1_slab(q, nb):
                k = piece_of(q)
                lo = q - RS_BOUNDS[k]
                asrc = agg1[k][:, lo * HID : (lo + nb) * HID]
                asb = pe1s.tile([P, AGB * HID], BF16, tag="asb")
                nc.sync.dma_start(out=asb[:, : nb * HID], in_=asrc)
                t0 = pe1s.tile([P, AGB * HID], BF16, tag="t0")
                nc.vector.tensor_tensor(
                    t0[:, : nb * HID],
                    asb[:, : nb * HID],
                    h1stage[:, q * HID : (q + nb) * HID],
                    op=ADD,
                )
                # relu(dis*(agg+h')) = dis*relu(agg+h'); the s1 scale on t0 is
                # cancelled by W2b = W2/s1, dis^2*s2 folded into dissqT
                r1 = pe1s.tile([P, AGB * HID], BF16, tag="r1")
                nc.scalar.activation(r1[:, : nb * HID], t0[:, : nb * HID], RELU)
                for j in range(nb):
                    b = q + j
                    pst = pe1p.tile([P, HID], BF16, tag="pst")
                    nc.tensor.transpose(
                        out=pst[:],
                        in_=r1[:, j * HID : (j + 1) * HID],
                        identity=idsb[:],
                    )
                    o1t = pe1.tile([P, HID], BF16, tag="o1t")
                    nc.scalar.copy(out=o1t[:], in_=pst[:])
                    ps2 = pe1q.tile([P, OUT_CH], F32, tag="ps2")
                    nc.tensor.matmul(
                        ps2[:], lhsT=o1t[:], rhs=w2sb[:], start=True, stop=True
                    )
                    nc.scalar.mul(
                        h2stage[:, b * OUT_CH : (b + 1) * OUT_CH],
                        ps2[:],
                        dissqsb[:, b : b + 1],
                    )
                    nc.vector.tensor_scalar_mul(
                        h2tab[:, b * OUT_CH : (b + 1) * OUT_CH],
                        ps2[:],
                        dissqsb[:, b : b + 1],
                    )

            def epi1_piece(k):
                for q in range(RS_BOUNDS[k], RS_BOUNDS[k + 1], AGB):
                    epi1_slab(q, min(AGB, RS_BOUNDS[k + 1] - q))
                lo, hi = RS_BOUNDS[k], RS_BOUNDS[k + 1]
                nc.sync.dma_start(
                    out=h2s[:, :].rearrange("(p b) f -> p b f", p=P)[
                        :, lo:hi, 0:OUT_CH
                    ],
                    in_=h2tab[:, lo * OUT_CH : hi * OUT_CH].rearrange(
                        "p (b f) -> p b f", f=OUT_CH
                    ),
                )

            def epi2_piece(k):
                for q in range(RS_BOUNDS[k], RS_BOUNDS[k + 1], AGB):
                    nb = min(AGB, RS_BOUNDS[k + 1] - q)
                    lo = q - RS_BOUNDS[k]
                    a2src = agg2[k][:, lo * OUT_CH : (lo + nb) * OUT_CH]
                    asb2 = pe2s.tile([P, AGB * OUT_CH], BF16, tag="asb2")
                    nc.sync.dma_start(out=asb2[:, : nb * OUT_CH], in_=a2src)
                    t2 = pe2s.tile([P, AGB * OUT_CH], BF16, tag="t2")
                    nc.vector.tensor_tensor(
                        t2[:, : nb * OUT_CH],
                        asb2[:, : nb * OUT_CH],
                        h2stage[:, q * OUT_CH : (q + nb) * OUT_CH],
                        op=ADD,
                    )
                    r2 = pe2s.tile([P, AGB * OUT_CH], F32, tag="r2")
                    nc.scalar.activation(
                        r2[:, : nb * OUT_CH], t2[:, : nb * OUT_CH], RELU
                    )
                    ob = pe2s.tile([P, AGB * OUT_CH], F32, tag="ob")
                    for j in range(nb):
                        b = q + j
                        nc.scalar.mul(
                            ob[:, j * OUT_CH : (j + 1) * OUT_CH],
                            r2[:, j * OUT_CH : (j + 1) * OUT_CH],
                            disosb[:, b : b + 1],
                        )
                    nc.sync.dma_start(
                        out=outY[:, q * OUT_CH : (q + nb) * OUT_CH],
                        in_=ob[:, : nb * OUT_CH],
                    )

            # ---- run the two sweeps (epilogue pools scoped per layer so
            # PSUM/SBUF budgets hold: L1 uses pgp(4)+pe1p(2)+pe1q(2) banks)
            agg_layer(h1s[:, 0:HID], HID, HID, GRP1, part1, agg1, epi1_piece)
            pe1q.release()
            pe1p.release()
            pe1.release()
            pe1s.release()

            pe2s = tc.alloc_tile_pool(name="pe2s", bufs=2)
            agg_layer(
                h2s[:, 0:OUT_CH], OUT_CH, OUT_CH, GRP2, part2, agg2, epi2_piece
            )
            pe2s.release()

    _spill_waits(nc)
    from concourse.library_overlay import lower_extended_insts

    lower_extended_insts(nc)
    return nc


def _prepare(x, src, dst):
    """Host-side: balanced block assignment + edge slot tables."""
    N = x.shape[0]
    assert N % NCORES == 0
    NPR = N // NCORES
    # blocks per core: keep (core, block) cells under ~245 edges on average
    E = src.shape[0]
    NB = max(
        int(math.ceil(NPR / P)),
        int(math.ceil(E / (NCORES * NCORES * P * 1.9))),
    )
    TB = NB * NCORES
    NPC = NB * P

    indeg = np.bincount(dst, minlength=N).astype(np.int64)
    dis = (1.0 / np.sqrt(indeg.astype(np.float32) + 1.0)).astype(np.float32)

    cs = (src // NPR).astype(np.int64)
    np.clip(cs, 0, NCORES - 1, out=cs)
    # v[d, c] = in-edges of d from core c
    v = np.bincount(dst * NCORES + cs, minlength=N * NCORES).reshape(N, NCORES)
    v = v.astype(np.int32)

    bl_of = np.empty(N, np.int32)  # block (within home core)
    lane_of = np.empty(N, np.int32)
    for k in range(NCORES):
        nodes = np.arange(k * NPR, (k + 1) * NPR)
        order = nodes[np.argsort(-indeg[nodes], kind="stable")]
        loads = np.zeros((NB, NCORES), np.int32)
        counts = np.zeros(NB, np.int32)
        full = np.zeros(NB, bool)
        for node in order:
            cand = (loads + v[node]).max(axis=1) + counts * 1e-4
            cand[full] = np.inf
            b = int(np.argmin(cand))
            bl_of[node] = b
            lane_of[node] = counts[b]
            loads[b] += v[node]
            counts[b] += 1
            if counts[b] >= P:
                full[b] = True

    home = np.arange(N) // NPR
    trow_of = lane_of * NB + bl_of  # local table row (lane-major)

    # per-(core, sweep-cell) counts -> shared chunk structure.
    # Sweep order is (bl, dst core): partials for low bl complete first on
    # every core, letting ReduceScatter pieces overlap the rest of the sweep.
    esw = bl_of[dst] * NCORES + home[dst]
    cellcnt = np.bincount(cs * TB + esw, minlength=NCORES * TB).reshape(NCORES, TB)
    k_b = np.maximum(1, -(-cellcnt.max(axis=0) // P))  # ceil(max/128), >=1
    chunk_start = np.zeros(TB + 1, np.int64)
    np.cumsum(k_b, out=chunk_start[1:])
    KTOT = int(chunk_start[-1])

    chunk_sw = np.repeat(np.arange(TB), k_b)
    idx_in_cell = np.arange(KTOT) - chunk_start[chunk_sw]
    chunk_first = idx_in_cell == 0
    chunk_last = idx_in_cell == (k_b[chunk_sw] - 1)

    # per-core slot tables
    esrc = np.zeros((NCORES, P, KTOT), np.int32)
    dstlA = np.full((NCORES, P, KTOT), PAD_LANE, np.float32)
    for k in range(NCORES):
        sel = np.where(cs == k)[0]
        eord = sel[np.argsort(esw[sel], kind="stable")]
        e_sw = esw[eord]
        # position within cell
        cnt = cellcnt[k]
        ofs = np.zeros(TB + 1, np.int64)
        np.cumsum(cnt, out=ofs[1:])
        pos = np.arange(eord.size) - ofs[e_sw]
        chunk = chunk_start[e_sw] + pos // P
        lane = pos % P
        tr = trow_of[src[eord]]
        esrc[k, lane, chunk] = tr
        dstlA[k, lane, chunk] = (lane_of[dst[eord]]).astype(np.float32)

    return dict(
        NB=NB, TB=TB, NPC=NPC, KTOT=KTOT, dis=dis,
        bl_of=bl_of, lane_of=lane_of, trow_of=trow_of,
        chunk_bl=(chunk_sw // NCORES).tolist(),
        chunk_c=(chunk_sw % NCORES).tolist(),
        chunk_first=chunk_first.tolist(),
        chunk_last=chunk_last.tolist(),
        esrc=esrc, dstlA=dstlA,
    )


def kernel(x, edge_index, W1, b1, W2, b2):
    x = np.ascontiguousarray(np.asarray(x, dtype=np.float32))
    W1 = np.asarray(W1, dtype=np.float32)
    W2 = np.asarray(W2, dtype=np.float32)
    b1 = np.asarray(b1, dtype=np.float32)
    b2 = np.asarray(b2, dtype=np.float32)
    assert not np.any(b1) and not np.any(b2), "zero-bias fast path only"
    src = np.asarray(edge_index[0]).astype(np.int64)
    dst = np.asarray(edge_index[1]).astype(np.int64)

    N, IN_CH = x.shape
    HID = W1.shape[1]
    OUT_CH = W2.shape[1]
    KT = IN_CH // P
    pr = _prepare(x, src, dst)
    NB, NPC, KTOT = pr["NB"], pr["NPC"], pr["KTOT"]
    NPR = N // NCORES

    nc = _build_program(
        NB, KTOT, pr["chunk_bl"], pr["chunk_c"], pr["chunk_first"],
        pr["chunk_last"], IN_CH, HID, OUT_CH,
    )

    W1b = np.zeros((P, KT * HID), np.float32)
    for k in range(KT):
        W1b[:, k * HID : (k + 1) * HID] = W1[k * P : (k + 1) * P, :]

    bl_of, lane_of = pr["bl_of"], pr["lane_of"]
    in_maps = []
    for c in range(NCORES):
        nodes = np.arange(c * NPR, (c + 1) * NPR)
        slot = bl_of[nodes] * P + lane_of[nodes]  # block-major slot in [0,NPC)
        xs = np.zeros((NPC, IN_CH), np.float32)
        xs[slot] = x[nodes]
        xT2 = np.zeros((P, KT * NPC), np.float32)
        for k in range(KT):
            xT2[:, k * NPC : (k + 1) * NPC] = xs[:, k * P : (k + 1) * P].T
        disc = np.ones((P, NB), np.float32)
        disc[lane_of[nodes], bl_of[nodes]] = pr["dis"][nodes]

        # idx1: wrapped-16 int16 per GC-chunk gather instruction
        esrc_c = pr["esrc"][c]  # [P, KTOT] lane, chunk
        flat = esrc_c.T.reshape(-1)  # [(chunk, lane)]
        idx1 = np.zeros((16, KTOT * 8), np.int16)
        for gi in range(0, KTOT, GC):
            nchunk = min(GC, KTOT - gi)
            seg = flat[gi * P : (gi + nchunk) * P]
            i = np.arange(seg.size)
            blockcols = np.zeros((16, nchunk * 8), np.int16)
            blockcols[i % 16, i // 16] = seg.astype(np.int16)
            idx1[:, gi * 8 : (gi + nchunk) * 8] = blockcols
        idx1 = np.tile(idx1, (8, 1))  # replicate across the 8 Q7 cores

        dstlA = pr["dstlA"][c]

        in_maps.append(
            {
                "xT2": xT2.astype(ml_dtypes.bfloat16),
                "W1b": W1b.astype(ml_dtypes.bfloat16),
                "W2b": (W2 / S1).astype(ml_dtypes.bfloat16),
                "disT": disc * S1,
                "dissqT": disc * disc * S2,
                "disO": disc / S2,
                "idx1": idx1,
                "dstlA": dstlA.astype(np.float32),
            }
        )

    res = run_bass_kernel_spmd(nc, in_maps, core_ids=list(range(NCORES)))
    global _last_results, _last_nc
    _last_results = res
    _last_nc = nc

    out = np.empty((N, OUT_CH), np.float32)
    for c in range(NCORES):
        oc = res.results[c]["outY"]  # [P, NB*OUT_CH]
        nodes = np.arange(c * NPR, (c + 1) * NPR)
        out[nodes] = oc[
            lane_of[nodes][:, None],
            (bl_of[nodes][:, None] * OUT_CH + np.arange(OUT_CH)[None, :]),
        ]
    return out


# revision 16
# speedup vs baseline: 1.0866x; 1.0325x over previous
"""2-layer GCN on 8 trn2 cores — src-partitioned with ReduceScatter.

Per layer (h' = (x@W)*dis, dis = (deg+1)^-1/2):
  1. Each core computes h' for its OWN 12500 nodes (x@W on PE), writes the
     local table (fp8-e3m4, scaled by a power of 2 to sit in e3m4's normal
     range; rows padded to 256B for the gather stride) to DRAM.
  2. Each core gathers h'[src] rows for its own ~200k out-edges (sorted by
     global dst block) with big merged dma_gather ops, and reduces them into
     per-dst-block partial sums via one-hot matmuls accumulated in PSUM.
     One-hot masks are built per chunk with tensor_scalar is_equal (runs in
     the DVE 4x_2p fast mode).
  3. Partial tables (bf16) are summed across cores with piecewise
     ReduceScatters that overlap the sweep; per-piece epilogues are issued
     late (after the RS landed) so they interleave with the sweep without
     stalling engine queues.
  4. Epilogue: out = relu(dis*(agg + h'_self)); fp8 scale factors are folded
     into host-provided dis vectors and W2, so no extra unscale ops exist.

Edges live on the core that owns their SOURCE node, so the gather reads are
all core-local (int16 row ids fit) and the only collectives are the
(cheap, output-sized) ReduceScatters.  Host-side, nodes are packed into
blocks with an 8-dim greedy balancer so each (core, dst-block) cell fits in
2 chunks of 128 edge slots.
"""

import math

import ml_dtypes
import numpy as np

import concourse.bass as bass
import concourse.mybir as mybir
import concourse.tile as tile
from concourse.bass_utils import run_bass_kernel_spmd
from concourse.library_config import mlp
from concourse.masks import make_identity
from concourse.vector_clock import ScopedClock

P = 128
NCORES = 8
F32 = mybir.dt.float32
BF16 = mybir.dt.bfloat16
FP8 = mybir.dt.float8e3  # e3m4
I32 = mybir.dt.int32
I16 = mybir.dt.int16
PAD_LANE = 1000.0
GC = 8  # chunks per dma_gather instruction (1024 idx = SWDGE cap)
S1 = 4.0  # layer-1 fp8 table scale (folded into disT and W2)
S2 = 16.0  # layer-2 fp8 table scale (folded into dissqT and disO)


def _patched_drain_and_barrier(self, tick_clock, wait_clock):
    # This walrus build rejects >1 sem wait on TPB_CTRL (Drain) instructions.
    drain_inst = self.nc.sync.drain()
    wait_clock.add_sem_waits(
        drain_inst.ins, ScopedClock({None: tick_clock.global_clock})
    )
    si = drain_inst.ins.sync_info
    waits = list(si.on_wait)
    if len(waits) > 1:
        while len(si.on_wait):
            si.on_wait.pop()
        si.on_wait.append(waits[0])
        for w in waits[1:]:
            d2 = self.nc.sync.drain(fusable=False)
            si2 = d2.ins.sync_info
            if si2 is None:
                d2.ins.sync_info = mybir.SyncInfo(on_wait=[w], on_update=[])
            else:
                si2.on_wait.append(w)
    self.nc.all_engine_barrier()
    popped = self.nc._tile_sem_poison_stack.pop()
    assert popped is self._sem_poison
    self.nc.clear_and_free_semaphores(list(self.sems.allocated().values()))
    self.nc.all_engine_barrier()


tile.TileContext._drain_and_barrier = _patched_drain_and_barrier


def _spill_waits(nc, max_waits=1):
    """Move extra sync waits onto dedicated single-wait NoOps (walrus limit)."""
    n = 0
    for f in nc.m.functions:
        for blk in f.blocks:
            il = blk.instructions
            out = []
            for inst in il:
                si = inst.sync_info
                if si is not None and len(si.on_wait) > max_waits:
                    waits = list(si.on_wait)
                    while len(si.on_wait):
                        si.on_wait.pop()
                    for w in waits[:max_waits]:
                        si.on_wait.append(w)
                    for w in waits[max_waits:]:
                        nop = mybir.InstNoOp(
                            name=f"waitspill-{n}",
                            sync_info=mybir.SyncInfo(on_wait=[w], on_update=[]),
                            bass_nofuse=True,
                            engine=inst.engine,
                        )
                        n += 1
                        out.append(nop)
                out.append(inst)
            blk.instructions = out
    return n


def _dma_gather_raw(
    gp, out_ap, in_ap, idxs_ap, num_idxs, num_idxs_reg, elem_size, elem_step,
    queue_num=0,
):
    """dma_gather (non-transpose, DRAM source) without the elem%256B assert:
    elem_step (row stride) must be a 256B multiple, elem_size may be a
    fraction of a row.  Descriptors move elem_size*dtype bytes each."""
    from concourse._compat import exact_div as _exact_div

    assert idxs_ap.dtype == mybir.dt.int16
    assert in_ap.dtype == out_ap.dtype
    stride_bytes = elem_step * mybir.dt.size(in_ap.dtype)
    stride_bytes_256 = _exact_div(stride_bytes, 256)
    _in_ap = gp.lower_ap_dma(in_ap, for_custom_bir_dma=True)
    _idxs_ap = gp.lower_ap(idxs_ap)
    _out_ap = gp.lower_ap(out_ap)
    return gp.add_instruction(
        mybir.InstDMAGatherAnt(
            name=gp.bass.get_next_instruction_name(),
            ins=[*_in_ap, _idxs_ap, gp.lower_val_access(gp.to_reg(num_idxs_reg))],
            outs=[_out_ap],
            transpose=False,
            num_idxs=num_idxs,
            elem_size=elem_size,
            stride_bytes_256=stride_bytes_256,
            gen_mode=0,
            single_packet=True,
            queue_num=queue_num,
            sbuf_tokens_per_rank=0,
            sbuf_free_dim_per_rank=0,
            sbuf_free_dim_pad_per_rank=0,
            sbuf_byte_offset=0,
        )
    )


def _build_program(NB, KTOT, chunk_bl, chunk_c, chunk_first, chunk_last, IN_CH, HID, OUT_CH):
    """SPMD program; per-core data comes via input tensors.

    Cells swept in (bl, dst-core) order; chunk_bl/chunk_c give each
    chunk's dst block (local id) and dst core; first/last flag cell
    boundaries (identical across cores)."""
    NPC = NB * P
    TB = NB * NCORES
    KT = IN_CH // P
    assert IN_CH % P == 0 and HID == P and OUT_CH * 2 == P
    IDXC = KTOT * 8  # idx table columns ( = KTOT*128/16 )
    ROWB = 256  # fp8 table row stride in bytes (gather stride granularity)

    nc = bass.Bass(num_swdge_queues=4)
    xT2 = nc.dram_tensor("xT2", [P, KT * NPC], BF16, kind="ExternalInput")
    W1b = nc.dram_tensor("W1b", [P, KT * HID], BF16, kind="ExternalInput")
    W2b = nc.dram_tensor("W2b", [P, OUT_CH], BF16, kind="ExternalInput")
    disT = nc.dram_tensor("disT", [P, NB], F32, kind="ExternalInput")  # s1*dis
    dissqT = nc.dram_tensor("dissqT", [P, NB], F32, kind="ExternalInput")  # s2*dis^2
    disO = nc.dram_tensor("disO", [P, NB], F32, kind="ExternalInput")  # dis/s2
    idx1 = nc.dram_tensor("idx1", [P, IDXC], I16, kind="ExternalInput")
    dstlA = nc.dram_tensor("dstlA", [P, KTOT], F32, kind="ExternalInput")
    outY = nc.dram_tensor("outY", [P, NB * OUT_CH], F32, kind="ExternalOutput")

    h1s = nc.dram_tensor("h1s", [NPC, ROWB], FP8)  # fp8 rows padded to 256B
    h2s = nc.dram_tensor("h2s", [NPC, ROWB], FP8)
    # partial/agg tables split by bl range so each ReduceScatter piece is a
    # contiguous tensor (walrus requires contiguous collective operands)
    RS_BOUNDS = [0, 32, 64, 88, NB]  # piece k covers bl [RS_BOUNDS[k], RS_BOUNDS[k+1])
    NPIECE = len(RS_BOUNDS) - 1
    PIECE_NB = [RS_BOUNDS[k + 1] - RS_BOUNDS[k] for k in range(NPIECE)]
    part1 = [
        nc.dram_tensor(f"part1_{k}", [NCORES, P * PIECE_NB[k] * HID], BF16)
        for k in range(NPIECE)
    ]
    part2 = [
        nc.dram_tensor(f"part2_{k}", [NCORES, P * PIECE_NB[k] * OUT_CH], BF16)
        for k in range(NPIECE)
    ]
    agg1 = [
        nc.dram_tensor(f"agg1_{k}", [P, PIECE_NB[k] * HID], BF16)
        for k in range(NPIECE)
    ]
    agg2 = [
        nc.dram_tensor(f"agg2_{k}", [P, PIECE_NB[k] * OUT_CH], BF16)
        for k in range(NPIECE)
    ]

    def piece_of(bl):
        for k in range(NPIECE):
            if bl < RS_BOUNDS[k + 1]:
                return k
        raise AssertionError(bl)

    rg = [list(range(NCORES))]
    RELU = mybir.ActivationFunctionType.Relu
    ADD = mybir.AluOpType.add
    ISEQ = mybir.AluOpType.is_equal

    cnt_regs = {}  # reused num_idxs registers (regfiles are tiny)

    def cnt_reg(eng, n):
        if (id(eng), n) not in cnt_regs:
            cnt_regs[(id(eng), n)] = eng.to_reg(n)
        return cnt_regs[(id(eng), n)]

    # per-chunk psum grouping: cells grouped by gb//GRP into one psum bank
    # pair; GRP is a multiple of NCORES so groups span whole dst blocks.
    GRP1 = 8  # cells per psum group in layer 1 (8*128 = 1024 f32 = 2 banks)
    GRP2 = 16  # cells per psum group in layer 2 (16*64 = 1024 f32 = 2 banks)
    SLAB_B = 4  # dst blocks per staging slab (x8 cores = 32 cells)
    EPI_DELAY = 34  # blocks between an RS firing and its epilogue being issued

    with tile.TileContext(nc) as tc:
        with tc.tile_pool(name="const", bufs=1) as cst:
            w1sb = cst.tile([P, KT * HID], BF16)
            nc.sync.dma_start(out=w1sb[:], in_=W1b[:, :])
            w2sb = cst.tile([P, OUT_CH], BF16)
            nc.sync.dma_start(out=w2sb[:], in_=W2b[:, :])
            dissb = cst.tile([P, NB], F32)
            nc.sync.dma_start(out=dissb[:], in_=disT[:, :])
            dissqsb = cst.tile([P, NB], F32)
            nc.sync.dma_start(out=dissqsb[:], in_=dissqT[:, :])
            disosb = cst.tile([P, NB], F32)
            nc.sync.dma_start(out=disosb[:], in_=disO[:, :])
            idx1sb = cst.tile([P, IDXC], I16)
            nc.sync.dma_start(out=idx1sb[:], in_=idx1[:, :])
            dstlAsb = cst.tile([P, KTOT], F32)
            nc.sync.dma_start(out=dstlAsb[:], in_=dstlA[:, :])
            iotasb = cst.tile([P, P], BF16)
            nc.gpsimd.iota(
                iotasb[:],
                pattern=[[1, P]],
                base=0,
                channel_multiplier=0,
                allow_small_or_imprecise_dtypes=True,
            )
            idsb = cst.tile([P, P], BF16)
            make_identity(nc, idsb[:])
            nc.gpsimd.load_library(mlp)

            # scaled-by-S1 self values (bf16) + fp8 table staging
            h1stage = cst.tile([P, NB * HID], BF16)
            h1tab = cst.tile([P, NB * HID], FP8)
            h2stage = cst.tile([P, NB * OUT_CH], BF16)
            h2tab = cst.tile([P, NB * OUT_CH], FP8)

            # ---- Phase A: h1' = (x @ W1) * (s1*dis)
            with (
                tc.tile_pool(name="pa", bufs=3) as pa,
                tc.tile_pool(name="pap", bufs=3, space="PSUM") as pap,
            ):
                XB = 4
                for q in range(0, NB, XB):
                    nb = min(XB, NB - q)
                    xt = pa.tile([P, KT * XB * P], BF16, tag="xt")
                    for k in range(KT):
                        nc.sync.dma_start(
                            out=xt[:, k * XB * P : k * XB * P + nb * P],
                            in_=xT2[:, k * NPC + q * P : k * NPC + (q + nb) * P],
                        )
                    for j in range(nb):
                        b = q + j
                        ps = pap.tile([P, HID], F32, tag="ps")
                        for k in range(KT):
                            nc.tensor.matmul(
                                ps[:],
                                lhsT=xt[:, k * XB * P + j * P : k * XB * P + (j + 1) * P],
                                rhs=w1sb[:, k * HID : (k + 1) * HID],
                                start=(k == 0),
                                stop=(k == KT - 1),
                            )
                        nc.scalar.mul(
                            h1stage[:, b * HID : (b + 1) * HID],
                            ps[:],
                            dissb[:, b : b + 1],
                        )
                        nc.vector.tensor_scalar_mul(
                            h1tab[:, b * HID : (b + 1) * HID],
                            ps[:],
                            dissb[:, b : b + 1],
                        )
            nc.sync.dma_start(
                out=h1s[:, :].rearrange("(p b) f -> p b f", p=P)[:, :, 0:HID],
                in_=h1tab[:].rearrange("p (b f) -> p b f", f=HID),
            )

            # ---- gather + one-hot reduce + piecewise RS, shared by layers
            RS_SPLITS = RS_BOUNDS[1:]  # bl boundaries at which RS pieces fire

            def agg_layer(table_ap, elem, F, GRP, part, agg, piece_epi):
                """Gather + one-hot reduce + partial writes, sweeping cells in
                (bl, dst-core) order; fires a ReduceScatter piece as the sweep
                passes each bl in RS_SPLITS.  piece_epi(k) is invoked (late,
                EPI_DELAY blocks after piece k's RS fired) so downstream
                epilogue work interleaves into the sweep without queue stalls."""
                rs_done = 0
                epi_done = 0
                assert GRP % NCORES == 0

                def part_view(bl):
                    # (tensor, local bl offset) for a given global bl
                    k = piece_of(bl)
                    return part[k], RS_BOUNDS[k], PIECE_NB[k]

                def emit_rs(upto_bl):
                    nonlocal rs_done
                    while rs_done < len(RS_SPLITS) and (
                        upto_bl >= RS_SPLITS[rs_done] + SLAB_B
                        or upto_bl > NB + 10_000
                    ):
                        nc.gpsimd.collective_compute(
                            "ReduceScatter",
                            ADD,
                            replica_groups=rg,
                            ins=[part[rs_done][:, :]],
                            outs=[agg[rs_done][:, :]],
                        )
                        rs_done += 1

                def emit_epi(upto_bl):
                    nonlocal epi_done
                    while epi_done < rs_done and (
                        upto_bl >= RS_SPLITS[epi_done] + EPI_DELAY
                        or upto_bl >= NB + 10_000
                    ):
                        piece_epi(epi_done)
                        epi_done += 1

                with (
                    tc.tile_pool(name="pg", bufs=4) as pg,
                    tc.tile_pool(name="pm", bufs=24) as pm,
                    tc.tile_pool(name="pst", bufs=2) as pstp,
                    tc.tile_pool(name="pgp", bufs=2, space="PSUM") as pgp,
                ):
                    pgrp = None
                    stag = None
                    slab0 = 0
                    for gq, gi in enumerate(range(0, KTOT, GC)):
                        nchunk = min(GC, KTOT - gi)
                        g = pg.tile([P, GC * elem], FP8, tag="g")
                        _dma_gather_raw(
                            nc.gpsimd,
                            g[:, : nchunk * elem].rearrange(
                                "p (c f) -> p c f", f=elem
                            ),
                            table_ap,
                            idx1sb[:, gi * 8 : (gi + nchunk) * 8],
                            nchunk * P,
                            cnt_reg(nc.gpsimd, nchunk * P),
                            elem,
                            ROWB,  # fp8: 256 elements = 256B row stride
                            queue_num=gq % 4,
                        )
                        for j in range(nchunk):
                            col = gi + j
                            bl = chunk_bl[col]
                            cc = chunk_c[col]
                            sw = bl * NCORES + cc
                            gsl = sw % GRP
                            if chunk_first[col] and gsl == 0:
                                pgrp = pgp.tile([P, GRP * F], F32, tag="pgrp")
                            if chunk_first[col] and sw % (NCORES * SLAB_B) == 0:
                                stag = pstp.tile(
                                    [P, SLAB_B * NCORES * F], BF16, tag="stag"
                                )
                                slab0 = bl
                            m = pm.tile([P, P], BF16, tag="m")
                            nc.vector.tensor_scalar(
                                m[:],
                                iotasb[:, :P],
                                dstlAsb[:, col : col + 1],
                                None,
                                op0=ISEQ,
                            )
                            nc.tensor.matmul(
                                pgrp[:, gsl * F : (gsl + 1) * F],
                                lhsT=m[:],
                                rhs=g[:, j * elem : j * elem + F],
                                start=chunk_first[col],
                                stop=chunk_last[col],
                            )
                            last_cell = sw == TB - 1
                            if chunk_last[col] and (gsl == GRP - 1 or last_cell):
                                # copy finished psum group -> staging (c-major)
                                nbsp = gsl // NCORES + 1  # blocks in this group
                                b0 = (bl - slab0) - (nbsp - 1)
                                nc.scalar.copy(
                                    out=stag.rearrange(
                                        "p (c b f) -> p c b f",
                                        c=NCORES,
                                        b=SLAB_B,
                                    )[:, :, b0 : b0 + nbsp, :],
                                    in_=pgrp[:, : (gsl + 1) * F].rearrange(
                                        "p (b c f) -> p c b f", c=NCORES, f=F
                                    ),
                                )
                            if chunk_last[col] and (
                                sw % (NCORES * SLAB_B) == NCORES * SLAB_B - 1
                                or last_cell
                            ):
                                nbl = bl - slab0 + 1
                                stv = stag.rearrange(
                                    "p (c b f) -> p c b f", c=NCORES, b=SLAB_B
                                )
                                pt, plo, pnb = part_view(slab0)
                                nc.sync.dma_start(
                                    out=pt[:, :].rearrange(
                                        "c (p b f) -> p c b f", p=P, f=F
                                    )[:, :, slab0 - plo : slab0 - plo + nbl, :],
                                    in_=stv[:, :, 0:nbl, :],
                                )
                                emit_rs(bl + 1)
                                emit_epi(bl + 1)
                    emit_rs(NB + 20_000)
                    emit_epi(NB + 20_000)

            # ---- Epilogue 1 (per RS piece) + dense layer 2 + h2 table write
            pe1s = tc.alloc_tile_pool(name="pe1s", bufs=2)
            pe1 = tc.alloc_tile_pool(name="pe1", bufs=8)
            pe1p = tc.alloc_tile_pool(name="pe1p", bufs=2, space="PSUM")
            pe1q = tc.alloc_tile_pool(name="pe1q", bufs=2, space="PSUM")
            AGB = 8

            def epi1_slab(q, nb):
                k = piece_of(q)
                lo = q - RS_BOUNDS[k]
                asrc = agg1[k][:, lo * HID : (lo + nb) * HID]
                asb = pe1s.tile([P, AGB * HID], BF16, tag="asb")
                nc.sync.dma_start(out=asb[:, : nb * HID], in_=asrc)
                t0 = pe1s.tile([P, AGB * HID], BF16, tag="t0")
                nc.vector.tensor_tensor(
                    t0[:, : nb * HID],
                    asb[:, : nb * HID],
                    h1stage[:, q * HID : (q + nb) * HID],
                    op=ADD,
                )
                # relu(dis*(agg+h')) = dis*relu(agg+h'); the s1 scale on t0 is
                # cancelled by W2b = W2/s1, dis^2*s2 folded into dissqT
                r1 = pe1s.tile([P, AGB * HID], BF16, tag="r1")
                nc.scalar.activation(r1[:, : nb * HID], t0[:, : nb * HID], RELU)
                for j in range(nb):
                    b = q + j
                    pst = pe1p.tile([P, HID], BF16, tag="pst")
                    nc.tensor.transpose(
                        out=pst[:],
                        in_=r1[:, j * HID : (j + 1) * HID],
                        identity=idsb[:],
                    )
                    o1t = pe1.tile([P, HID], BF16, tag="o1t")
                    nc.scalar.copy(out=o1t[:], in_=pst[:])
                    ps2 = pe1q.tile([P, OUT_CH], F32, tag="ps2")
                    nc.tensor.matmul(
                        ps2[:], lhsT=o1t[:], rhs=w2sb[:], start=True, stop=True
                    )
                    nc.scalar.mul(
                        h2stage[:, b * OUT_CH : (b + 1) * OUT_CH],
                        ps2[:],
                        dissqsb[:, b : b + 1],
                    )
                    nc.vector.tensor_scalar_mul(
                        h2tab[:, b * OUT_CH : (b + 1) * OUT_CH],
                        ps2[:],
                        dissqsb[:, b : b + 1],
                    )

            def epi1_piece(k):
                for q in range(RS_BOUNDS[k], RS_BOUNDS[k + 1], AGB):
                    epi1_slab(q, min(AGB, RS_BOUNDS[k + 1] - q))
                lo, hi = RS_BOUNDS[k], RS_BOUNDS[k + 1]
                nc.sync.dma_start(
                    out=h2s[:, :].rearrange("(p b) f -> p b f", p=P)[
                        :, lo:hi, 0:OUT_CH
                    ],
                    in_=h2tab[:, lo * OUT_CH : hi * OUT_CH].rearrange(
                        "p (b f) -> p b f", f=OUT_CH
                    ),
                )

            def epi2_piece(k):
                for q in range(RS_BOUNDS[k], RS_BOUNDS[k + 1], AGB):
                    nb = min(AGB, RS_BOUNDS[k + 1] - q)
                    lo = q - RS_BOUNDS[k]
                    a2src = agg2[k][:, lo * OUT_CH : (lo + nb) * OUT_CH]
                    asb2 = pe2s.tile([P, AGB * OUT_CH], BF16, tag="asb2")
                    nc.sync.dma_start(out=asb2[:, : nb * OUT_CH], in_=a2src)
                    t2 = pe2s.tile([P, AGB * OUT_CH], BF16, tag="t2")
                    nc.vector.tensor_tensor(
                        t2[:, : nb * OUT_CH],
                        asb2[:, : nb * OUT_CH],
                        h2stage[:, q * OUT_CH : (q + nb) * OUT_CH],
                        op=ADD,
                    )
                    r2 = pe2s.tile([P, AGB * OUT_CH], F32, tag="r2")
                    nc.scalar.activation(
                        r2[:, : nb * OUT_CH], t2[:, : nb * OUT_CH], RELU
                    )
                    ob = pe2s.tile([P, AGB * OUT_CH], F32, tag="ob")
                    for j in range(nb):
                        b = q + j
                        nc.scalar.mul(
                            ob[:, j * OUT_CH : (j + 1) * OUT_CH],
                            r2[:, j * OUT_CH : (j + 1) * OUT_CH],
                            disosb[:, b : b + 1],
                        )
                    nc.sync.dma_start(
                        out=outY[:, q * OUT_CH : (q + nb) * OUT_CH],
                        in_=ob[:, : nb * OUT_CH],
                    )

            # ---- run the two sweeps (epilogue pools scoped per layer so
            # PSUM/SBUF budgets hold: L1 uses pgp(4)+pe1p(2)+pe1q(2) banks)
            agg_layer(h1s[:, 0:HID], HID, HID, GRP1, part1, agg1, epi1_piece)
            pe1q.release()
            pe1p.release()
            pe1.release()
            pe1s.release()

            pe2s = tc.alloc_tile_pool(name="pe2s", bufs=2)
            agg_layer(
                h2s[:, 0:OUT_CH], OUT_CH, OUT_CH, GRP2, part2, agg2, epi2_piece
            )
            pe2s.release()

    _spill_waits(nc)
    from concourse.library_overlay import lower_extended_insts

    lower_extended_insts(nc)
    return nc


def _prepare(x, src, dst):
    """Host-side: balanced block assignment + edge slot tables."""
    N = x.shape[0]
    assert N % NCORES == 0
    NPR = N // NCORES
    # blocks per core: keep (core, block) cells under ~245 edges on average
    E = src.shape[0]
    NB = max(
        int(math.ceil(NPR / P)),
        int(math.ceil(E / (NCORES * NCORES * P * 1.9))),
    )
    TB = NB * NCORES
    NPC = NB * P

    indeg = np.bincount(dst, minlength=N).astype(np.int64)
    dis = (1.0 / np.sqrt(indeg.astype(np.float32) + 1.0)).astype(np.float32)

    cs = (src // NPR).astype(np.int64)
    np.clip(cs, 0, NCORES - 1, out=cs)
    # v[d, c] = in-edges of d from core c
    v = np.bincount(dst * NCORES + cs, minlength=N * NCORES).reshape(N, NCORES)
    v = v.astype(np.int32)

    bl_of = np.empty(N, np.int32)  # block (within home core)
    lane_of = np.empty(N, np.int32)
    for k in range(NCORES):
        nodes = np.arange(k * NPR, (k + 1) * NPR)
        order = nodes[np.argsort(-indeg[nodes], kind="stable")]
        loads = np.zeros((NB, NCORES), np.int32)
        counts = np.zeros(NB, np.int32)
        full = np.zeros(NB, bool)
        for node in order:
            cand = (loads + v[node]).max(axis=1) + counts * 1e-4
            cand[full] = np.inf
            b = int(np.argmin(cand))
            bl_of[node] = b
            lane_of[node] = counts[b]
            loads[b] += v[node]
            counts[b] += 1
            if counts[b] >= P:
                full[b] = True

    home = np.arange(N) // NPR
    trow_of = lane_of * NB + bl_of  # local table row (lane-major)

    # per-(core, sweep-cell) counts -> shared chunk structure.
    # Sweep order is (bl, dst core): partials for low bl complete first on
    # every core, letting ReduceScatter pieces overlap the rest of the sweep.
    esw = bl_of[dst] * NCORES + home[dst]
    cellcnt = np.bincount(cs * TB + esw, minlength=NCORES * TB).reshape(NCORES, TB)
    k_b = np.maximum(1, -(-cellcnt.max(axis=0) // P))  # ceil(max/128), >=1
    chunk_start = np.zeros(TB + 1, np.int64)
    np.cumsum(k_b, out=chunk_start[1:])
    KTOT = int(chunk_start[-1])

    chunk_sw = np.repeat(np.arange(TB), k_b)
    idx_in_cell = np.arange(KTOT) - chunk_start[chunk_sw]
    chunk_first = idx_in_cell == 0
    chunk_last = idx_in_cell == (k_b[chunk_sw] - 1)

    # per-core slot tables
    esrc = np.zeros((NCORES, P, KTOT), np.int32)
    dstlA = np.full((NCORES, P, KTOT), PAD_LANE, np.float32)
    for k in range(NCORES):
        sel = np.where(cs == k)[0]
        eord = sel[np.argsort(esw[sel], kind="stable")]
        e_sw = esw[eord]
        # position within cell
        cnt = cellcnt[k]
        ofs = np.zeros(TB + 1, np.int64)
        np.cumsum(cnt, out=ofs[1:])
        pos = np.arange(eord.size) - ofs[e_sw]
        chunk = chunk_start[e_sw] + pos // P
        lane = pos % P
        tr = trow_of[src[eord]]
        esrc[k, lane, chunk] = tr
        dstlA[k, lane, chunk] = (lane_of[dst[eord]]).astype(np.float32)

    return dict(
        NB=NB, TB=TB, NPC=NPC, KTOT=KTOT, dis=dis,
        bl_of=bl_of, lane_of=lane_of, trow_of=trow_of,
        chunk_bl=(chunk_sw // NCORES).tolist(),
        chunk_c=(chunk_sw % NCORES).tolist(),
        chunk_first=chunk_first.tolist(),
        chunk_last=chunk_last.tolist(),
        esrc=esrc, dstlA=dstlA,
    )


def kernel(x, edge_index, W1, b1, W2, b2):
    x = np.ascontiguousarray(np.asarray(x, dtype=np.float32))
    W1 = np.asarray(W1, dtype=np.float32)
    W2 = np.asarray(W2, dtype=np.float32)
    b1 = np.asarray(b1, dtype=np.float32)
    b2 = np.asarray(b2, dtype=np.float32)
    assert not np.any(b1) and not np.any(b2), "zero-bias fast path only"
    src = np.asarray(edge_index[0]).astype(np.int64)
    dst = np.asarray(edge_index[1]).astype(np.int64)

    N, IN_CH = x.shape
    HID = W1.shape[1]
    OUT_CH = W2.shape[1]
    KT = IN_CH // P
    pr = _prepare(x, src, dst)
    NB, NPC, KTOT = pr["NB"], pr["NPC"], pr["KTOT"]
    NPR = N // NCORES

    nc = _build_program(
        NB, KTOT, pr["chunk_bl"], pr["chunk_c"], pr["chunk_first"],
        pr["chunk_last"], IN_CH, HID, OUT_CH,
    )

    W1b = np.zeros((P, KT * HID), np.float32)
    for k in range(KT):
        W1b[:, k * HID : (k + 1) * HID] = W1[k * P : (k + 1) * P, :]

    bl_of, lane_of = pr["bl_of"], pr["lane_of"]
    in_maps = []
    for c in range(NCORES):
        nodes = np.arange(c * NPR, (c + 1) * NPR)
        slot = bl_of[nodes] * P + lane_of[nodes]  # block-major slot in [0,NPC)
        xs = np.zeros((NPC, IN_CH), np.float32)
        xs[slot] = x[nodes]
        xT2 = np.zeros((P, KT * NPC), np.float32)
        for k in range(KT):
            xT2[:, k * NPC : (k + 1) * NPC] = xs[:, k * P : (k + 1) * P].T
        disc = np.ones((P, NB), np.float32)
        disc[lane_of[nodes], bl_of[nodes]] = pr["dis"][nodes]

        # idx1: wrapped-16 int16 per GC-chunk gather instruction
        esrc_c = pr["esrc"][c]  # [P, KTOT] lane, chunk
        flat = esrc_c.T.reshape(-1)  # [(chunk, lane)]
        idx1 = np.zeros((16, KTOT * 8), np.int16)
        for gi in range(0, KTOT, GC):
            nchunk = min(GC, KTOT - gi)
            seg = flat[gi * P : (gi + nchunk) * P]
            i = np.arange(seg.size)
            blockcols = np.zeros((16, nchunk * 8), np.int16)
            blockcols[i % 16, i // 16] = seg.astype(np.int16)
            idx1[:, gi * 8 : (gi + nchunk) * 8] = blockcols
        idx1 = np.tile(idx1, (8, 1))  # replicate across the 8 Q7 cores

        dstlA = pr["dstlA"][c]

        in_maps.append(
            {
                "xT2": xT2.astype(ml_dtypes.bfloat16),
                "W1b": W1b.astype(ml_dtypes.bfloat16),
                "W2b": (W2 / S1).astype(ml_dtypes.bfloat16),
                "disT": disc * S1,
                "dissqT": disc * disc * S2,
                "disO": disc / S2,
                "idx1": idx1,
                "dstlA": dstlA.astype(np.float32),
            }
        )

    res = run_bass_kernel_spmd(nc, in_maps, core_ids=list(range(NCORES)))
    global _last_results, _last_nc
    _last_results = res
    _last_nc = nc

    out = np.empty((N, OUT_CH), np.float32)
    for c in range(NCORES):
        oc = res.results[c]["outY"]  # [P, NB*OUT_CH]
        nodes = np.arange(c * NPR, (c + 1) * NPR)
        out[nodes] = oc[
            lane_of[nodes][:, None],
            (bl_of[nodes][:, None] * OUT_CH + np.arange(OUT_CH)[None, :]),
        ]
    return out
